# revision 29
# baseline (speedup 1.0000x reference)
"""Trainium2 Bass kernel for memory-augmented causal attention.

Reference computation (b=2, n=1024, m=1024 memory, 16 heads, d_head=64):
  q = (x @ Wq) * scale ; k,v = split(x @ Wkv) ; k = [mem_k; k] ; v = [mem_v; v]
  sim = q k^T + pos_bias ; causal mask on self part ; softmax ; out = attn v
  return out @ Wo + bo

Sharding: 16 heads across 8 cores (2 heads/core), both batches on every core.
Each core computes a partial output (its heads' contribution through Wo rows);
host sums the 8 partials.

Softmax bias handling: exp(sim + bias) = exp(sim) * exp(bias).  The host
precomputes ebias = exp(pos_bias) with masked entries set to 0 (which also
applies the causal mask), f16.  ScalarE does a shifted exp of the raw logits
(PSUM->SBUF, exp(x-4) so masked-but-large logits can't hit f16 inf before
the zeroing multiply), then DVE multiplies by the ebias tile.

Softmax denominators come from a ones-column appended to V (row 64 of the
AV accumulation); all four (h, b) AV accumulators live in one [65, 2048]
PSUM tile so the whole normalization runs as a few wide DVE ops.
All matmuls are fp16 (full-rate on the TRN2 PE).
"""

import numpy as np

import concourse.bass as bass
import concourse.mybir as mybir
import concourse.tile as tile
from concourse import bacc
from concourse import bass_utils
from concourse.masks import make_identity

F32 = mybir.dt.float32
BF16 = mybir.dt.bfloat16
F16 = mybir.dt.float16

HEADS = 16
DH = 64               # head dim
B = 2                 # batch
N = 1024              # query length
M = 1024              # memory length
JT = N + M            # total key length
DIM = 1024
SCALE = DH ** -0.5
NCORE = 8
HPC = HEADS // NCORE  # heads per core = 2

NKC = DIM // 128      # contraction chunks for projections = 8
NJ = JT // 128        # j chunks = 16
NJ_MEM = M // 128     # memory j chunks = 8
NIC = N // 512        # i chunks of 512 = 2


def _self_chunks(ic):
    # self j-chunk k (j0 = 1024 + 128k) unmasked for i-chunk ic iff
    # j0 <= 1023 + ic*512 + 1024  ->  128k <= ic*512 + 511
    return (ic * 512 + 511) // 128 + 1


def _unmasked_jcs(ic):
    return list(range(NJ_MEM)) + [NJ_MEM + k for k in range(min(8, _self_chunks(ic)))]


_NC_CACHE = None


def _build():
    global _NC_CACHE
    if _NC_CACHE is not None:
        return _NC_CACHE

    nc = bacc.Bacc("TRN2", target_bir_lowering=False, debug=False)

    XT = nc.dram_tensor("xT", [B, DIM, N], F16, kind="ExternalInput").ap()
    WQ = nc.dram_tensor("wq", [DIM, 128], F16, kind="ExternalInput").ap()
    WK = nc.dram_tensor("wk", [DIM, 128], F16, kind="ExternalInput").ap()
    WV = nc.dram_tensor("wv", [DIM, 128], F16, kind="ExternalInput").ap()
    WO = nc.dram_tensor("wo", [128, DIM], F16, kind="ExternalInput").ap()
    MKT = nc.dram_tensor("mkT", [B, 128, M], F16, kind="ExternalInput").ap()
    MV = nc.dram_tensor("mv", [B, HPC, NJ_MEM, 128, DH + 1], F16,
                        kind="ExternalInput").ap()
    EBIAS = nc.dram_tensor("ebias", [HPC, JT, N], F16, kind="ExternalInput").ap()
    ONES = nc.dram_tensor("ones_self", [128, HPC, NJ - NJ_MEM, 1], F16,
                          kind="ExternalInput").ap()
    OUT = nc.dram_tensor("out", [B, N, DIM], F16, kind="ExternalOutput").ap()

    with tile.TileContext(nc) as tc:
        with tc.tile_pool(name="const", bufs=1) as cp, \
             tc.tile_pool(name="wts", bufs=1) as wp, \
             tc.tile_pool(name="xtp", bufs=16) as xtp, \
             tc.tile_pool(name="big", bufs=1) as bigp, \
             tc.tile_pool(name="stage", bufs=2) as stp, \
             tc.tile_pool(name="biasp", bufs=32) as biasp, \
             tc.tile_pool(name="expp", bufs=16) as expp, \
             tc.tile_pool(name="outst", bufs=4) as outstp, \
             tc.tile_pool(name="smalls", bufs=2) as smallp, \
             tc.tile_pool(name="psum", bufs=1, space="PSUM") as psp:

            # ---- constants ----
            identf = cp.tile([128, 128], F32)
            make_identity(nc, identf[:])
            identh = cp.tile([128, 128], F16)
            nc.vector.tensor_copy(identh[:], identf[:])
            # per-partition bias vector for the shifted exp
            negc = cp.tile([128, 1], F32, name="negc")
            nc.gpsimd.memset(negc[:], -4.0)

            # ---- weights (scalar queue; sync busy with xT) ----
            wq_t = wp.tile([128, NKC * 128], F16, tag="wqo")
            wk_t = wp.tile([128, NKC * 128], F16)
            wv_t = wp.tile([128, NKC * 128], F16)
            wo_t = wp.tile([128, DIM], F16, tag="wqo")
            def load_w(tl, src):
                nc.scalar.dma_start(
                    tl[:].rearrange("p (kc m) -> p kc m", m=128),
                    src.rearrange("(kc p) m -> p kc m", p=128))
            load_w(wq_t, WQ)

            # ---- persistent per-batch tensors ----
            qT = [bigp.tile([128, N], F16, name=f"qT{b}") for b in range(B)]
            kT = [bigp.tile([128, JT], F16, name=f"kT{b}") for b in range(B)]
            vaug = [bigp.tile([128, HPC * NJ * (DH + 1)], F16, name=f"vaug{b}")
                    for b in range(B)]

            def vaug_slice(b, h, jc):
                o = (h * NJ + jc) * (DH + 1)
                return vaug[b][:, o:o + DH + 1]
            outT = [bigp.tile([128, N], F16, name=f"outT{b}") for b in range(B)]

            def warm(n, width=128):
                wps = psp.tile([128, width], F32, name="warmps", tag="simps",
                               bufs=2)
                for _ in range(n):
                    nc.tensor.matmul(wps[:, 0:128], identh[:], identh[:],
                                     start=True, stop=True,
                                     skip_group_check=True)

            # preload all xT tiles for both batches (sync + scalar queues);
            xts = {}
            for b in range(B):
                for kc in range(NKC):
                    t = xtp.tile([128, N], F16, name=f"xt{b}_{kc}", tag="xt")
                    eng = nc.sync if (kc % 2 == 0) else nc.scalar
                    eng.dma_start(t[:], XT[b, kc * 128:(kc + 1) * 128, :])
                    xts[(b, kc)] = t
                if b == 0:
                    load_w(wk_t, WK)
            load_w(wv_t, WV)
            nc.scalar.dma_start(wo_t[:], WO)

            for b in range(B):
                # mem parts straight from DRAM
                nc.sync.dma_start(kT[b][:, 0:M], MKT[b])
                for h in range(HPC):
                    nc.gpsimd.dma_start(
                        vaug[b][:].rearrange(
                            "p (h jc x) -> p h jc x", h=HPC, x=DH + 1)[:, h, 0:NJ_MEM],
                        MV[b, h].rearrange("jc p x -> p jc x"))
                # ones columns for the self chunks, via strided DMA
                for h in range(HPC):
                    nc.gpsimd.dma_start(
                        vaug[b][:].rearrange(
                            "p (s x) -> p s x", x=DH + 1)[
                            :, h * NJ + NJ_MEM:h * NJ + NJ, DH:DH + 1],
                        ONES[:, h])

            # =============== Phase 1: projections ===============
            # ScalarE is idle until the first exp, so all projection-phase
            # PSUM evacuations go through it, keeping DVE free for the
            # attention-phase ebias multiplies.
            def proj_qk(kind, b):
                wt = wq_t if kind == "q" else wk_t
                ps = psp.tile([128, N], F32, name="projps", tag="simps", bufs=2)
                for icx in range(NIC):
                    for kc in range(NKC):
                        nc.tensor.matmul(
                            ps[:, icx * 512:(icx + 1) * 512],
                            wt[:, kc * 128:(kc + 1) * 128],
                            xts[(b, kc)][:, icx * 512:(icx + 1) * 512],
                            start=(kc == 0), stop=(kc == NKC - 1))
                if kind == "q":
                    nc.scalar.copy(qT[b][:], ps[:])
                else:
                    # vector: keeps the ScalarE queue clear so the first exps
                    # aren't serialized behind prologue copies
                    nc.vector.tensor_copy(kT[b][:, M:JT], ps[:])

            def proj_v(b):
                vst = stp.tile([128, N], F16, name="vstage")
                ps = psp.tile([128, N], F32, name="vps", tag="simps", bufs=2)
                for icx in range(NIC):
                    for kc in range(NKC):
                        nc.tensor.matmul(
                            ps[:, icx * 512:(icx + 1) * 512],
                            wv_t[:, kc * 128:(kc + 1) * 128],
                            xts[(b, kc)][:, icx * 512:(icx + 1) * 512],
                            start=(kc == 0), stop=(kc == NKC - 1))
                nc.vector.tensor_copy(vst[:], ps[:])
                tps = psp.tile([128, 8 * 128], F16, name="tps", tag="simps",
                               bufs=2)
                for jb in range(8):
                    nc.tensor.transpose(
                        tps[:, jb * 128:(jb + 1) * 128],
                        vst[:, jb * 128:(jb + 1) * 128], identh[:])
                # one fused copy after all 8 transposes (avoids PE-write /
                # ScalarE-read overlap on the shared PSUM bank)
                dst = vaug[b][:].rearrange(
                    "p (h jjc x) -> p jjc h x", h=HPC, x=DH + 1)[
                    :, NJ_MEM:NJ, :, 0:DH]
                src = tps[:].rearrange("p (jb h x) -> p jb h x", jb=8, x=DH)
                nc.vector.tensor_copy(dst, src)

            warm(8)
            for kind, b in (("q", 0), ("q", 1), ("k", 0), ("k", 1)):
                proj_qk(kind, b)
            proj_v(0)
            proj_v(1)

            # =============== Phase 2: attention ===============
            # out-projection runs at the tail for both i-chunks: during
            # attention all 8 PSUM banks belong to the sims pipeline (4) and
            # the AV accumulators (4)
            opj_idx = [0]

            def out_proj_half(b, ib, allow_avps=True):
                ob = outstp.tile([128, DIM], F16, name="ob")
                for dc in range(DIM // 512):
                    # rotate across both psum tags (avh is dead by now) for
                    # effective depth 3: keeps the tail MM stream gap-free
                    opj_idx[0] += 1
                    if allow_avps and opj_idx[0] % 3 == 0:
                        ps = psp.tile([128, 512], F32, name="ops2",
                                      tag="avps", bufs=1)
                    else:
                        ps = psp.tile([128, 512], F32, name="ops", tag="simps",
                                      bufs=2)
                    nc.tensor.matmul(
                        ps[:],
                        outT[b][:, ib * 128:(ib + 1) * 128],
                        wo_t[:, dc * 512:(dc + 1) * 512],
                        start=True, stop=True)
                    if dc % 2 == 0:
                        nc.scalar.copy(ob[:, dc * 512:(dc + 1) * 512], ps[:])
                    else:
                        nc.vector.tensor_copy(ob[:, dc * 512:(dc + 1) * 512],
                                              ps[:])
                nc.sync.dma_start(OUT[b, ib * 128:(ib + 1) * 128, :], ob[:])

            epilogue = []  # deferred normalization work from the previous ic

            for ic in range(NIC):
                jcs = _unmasked_jcs(ic)
                avh = []  # single [65, 2048] accumulator, cols = (h, b, 512)
                pend = [[] for _ in range(HPC)]  # AV one iteration behind

                def do_av(h, p, last, avh=avh):
                    if not avh:
                        avh.append(psp.tile([DH + 1, B * HPC * 512], F32,
                                            name="av", tag="avps", bufs=1))
                    expt_, jc_, idx = p
                    for b in range(B):
                        o = (h * B + b) * 512
                        nc.tensor.matmul(
                            avh[0][:, o:o + 512],
                            vaug_slice(b, h, jc_),
                            expt_[:, b * 512:(b + 1) * 512],
                            start=(idx == 0), stop=last,
                            skip_group_check=True)

                for jj, jc in enumerate(jcs):
                    ebts = []
                    simps = []
                    for h in range(HPC):
                        ebt = biasp.tile([128, 512], F16, name=f"ebt{h}",
                                         tag="ebt")
                        # all ebias tiles on the sync queue: routing them via
                        # gpsimd couples them behind the (stalling)
                        # partition_broadcast in that FIFO and starves the AVs
                        nc.sync.dma_start(
                            ebt[:],
                            EBIAS[h, jc * 128:(jc + 1) * 128,
                                  ic * 512:(ic + 1) * 512])
                        ebts.append(ebt)
                        simps.append(psp.tile([128, N], F32, name=f"simps{h}",
                                              tag="simps", bufs=2))
                    # sims b-outer: consecutive MMs alternate 64-row groups
                    # (h0 at partitions 0-63, h1 at 64-127) so pairs can run
                    # concurrently as row-tiled matmuls
                    def sim_mm(b, h):
                        nc.tensor.matmul(
                            simps[h][:, b * 512:(b + 1) * 512],
                            kT[b][h * 64:(h + 1) * 64,
                                  jc * 128:(jc + 1) * 128],
                            qT[b][h * 64:(h + 1) * 64,
                                  ic * 512:(ic + 1) * 512],
                            start=True, stop=True, skip_group_check=True)
                    for b in range(B):
                        for h in range(HPC):
                            sim_mm(b, h)
                    # idempotent duplicates (start=True rewrites the same
                    # slice): filler PE work that closes the per-unit idle
                    # gaps so the HAM clock gate stays at 2.4 GHz
                    sim_mm(0, 0)
                    sim_mm(1, 1)
                    for h in range(HPC):
                        expr = expp.tile([128, N], F16, name="expr",
                                         tag="expr", bufs=3)
                        nc.scalar.activation(
                            expr[:], simps[h][:],
                            mybir.ActivationFunctionType.Exp, bias=negc[:])
                        expt = expp.tile([128, N], F16, name="expt",
                                         tag="expt")
                        for b in range(B):
                            nc.vector.tensor_tensor(
                                expt[:, b * 512:(b + 1) * 512],
                                expr[:, b * 512:(b + 1) * 512],
                                ebts[h][:], mybir.AluOpType.mult)
                        q = pend[h]
                        q.append((expt, jc, jj))
                        if jj == len(jcs) - 1:
                            # drain fully in the last unit so the ic boundary
                            # has no trailing AV burst stalling the next ic's
                            # sims (and thus the exp stream)
                            while q:
                                do_av(h, q.pop(0), len(q) == 0)
                        else:
                            # during the previous ic's deferred normalization
                            # (a DVE burst that delays the ebias multiplies)
                            # hold AVs back so they can't head-of-line block
                            # the PE FIFO waiting on those multiplies
                            depth = 4 if (ic > 0 and jj < 6) else 1
                            while len(q) > depth:
                                do_av(h, q.pop(0), False)
                    if epilogue and jj >= 1:
                        # previous ic's deferred normalization: runs on DVE /
                        # gpsimd under the ScalarE-bound exp stream
                        epilogue.pop(0)()
                for h in range(HPC):
                    q = pend[h]
                    while q:
                        do_av(h, q.pop(0), len(q) == 0)

                def normalize(b, ic=ic, avh=avh):
                    # av columns are (h, b, 512); batch b's two head slices
                    # via a strided 3d AP so one chain normalizes them both
                    wide = HPC * 512

                    def bsl(src, p0, p1):
                        return src[p0:p1].rearrange(
                            "p (h b f) -> p b h f", h=HPC, b=B)[:, b]
                    tg = f"n{b}"
                    sums_sb = smallp.tile([1, wide], F32, name="sums_sb",
                                          tag=tg)
                    nc.vector.tensor_copy(sums_sb[:].rearrange(
                        "p (h f) -> p h f", h=HPC), bsl(avh[0], DH, DH + 1))
                    recip = smallp.tile([1, wide], F32, name="recip", tag=tg)
                    nc.vector.reciprocal_approx_fast(recip[:], sums_sb[:])
                    recipb = smallp.tile([DH, wide], F32, name="recipb",
                                         tag=tg)
                    nc.gpsimd.partition_broadcast(recipb[:], recip[:])
                    nstage = smallp.tile([DH, wide], F16, name="nstage",
                                         tag=tg)
                    nc.vector.tensor_tensor(
                        nstage[:].rearrange("p (h f) -> p h f", h=HPC),
                        bsl(avh[0], 0, DH), recipb[:].rearrange(
                            "p (h f) -> p h f", h=HPC),
                        mybir.AluOpType.mult)
                    for h in range(HPC):
                        nc.sync.dma_start(
                            outT[b][h * 64:(h + 1) * 64,
                                    ic * 512:(ic + 1) * 512],
                            nstage[:, h * 512:(h + 1) * 512])

                if ic < NIC - 1:
                    epilogue.append(lambda f=normalize: f(0))
                    epilogue.append(lambda f=normalize: f(1))
                else:
                    # tail: the ic0 output columns (ib 0-3) are already
                    # normalized, so their projection overlaps the final
                    # normalization chains on DVE/gpsimd
                    for b in range(B):
                        for ib in range(4):
                            out_proj_half(b, ib, allow_avps=False)
                    normalize(0)
                    normalize(1)

            # =============== Phase 3: output projection (tail) ===============
            for b in range(B):
                for ib in range(4, 8):
                    out_proj_half(b, ib)

    nc.compile()
    _NC_CACHE = nc
    return nc


def _prep_inputs(x, mem_k, mem_v, pos_bias, Wq, Wkv, Wo):
    """Build per-core input maps (host-side sharding)."""
    x = np.ascontiguousarray(x, dtype=np.float32)
    xT = np.ascontiguousarray(x.transpose(0, 2, 1)).astype(np.float16)

    # exp(bias) with causal mask folded in as zeros: [16, JT, N] f16
    pb = np.ascontiguousarray(
        pos_bias[0].transpose(0, 2, 1)).astype(np.float32)     # [16, JT, N]
    jj = np.arange(JT)[:, None]
    ii = np.arange(N)[None, :]
    mask = jj > (ii + M)
    eb = np.where(mask[None], np.float32(0.0), np.exp(pb)).astype(np.float16)

    ones = np.ones((B, NJ_MEM, 128, 1), dtype=np.float16)
    in_maps = []
    for c in range(NCORE):
        cs = 128 * c
        wq = np.ascontiguousarray(Wq[:, cs:cs + 128] * SCALE).astype(np.float16)
        wk = np.ascontiguousarray(Wkv[:, cs:cs + 128]).astype(np.float16)
        wv = np.ascontiguousarray(Wkv[:, DIM + cs:DIM + cs + 128]).astype(np.float16)
        wo = np.ascontiguousarray(Wo[cs:cs + 128, :]).astype(np.float16)
        mkT = np.ascontiguousarray(
            mem_k[:, :, cs:cs + 128].transpose(0, 2, 1)).astype(np.float16)
        mv_s = mem_v[:, :, cs:cs + 128].astype(np.float16).reshape(B, NJ_MEM, 128, 2, DH)
        mv = np.empty((B, HPC, NJ_MEM, 128, DH + 1), dtype=np.float16)
        for h in range(HPC):
            mv[:, h, :, :, 0:DH] = mv_s[:, :, :, h, :]
            mv[:, h, :, :, DH:] = ones
        ebias = np.ascontiguousarray(eb[2 * c:2 * c + 2])
        in_maps.append({
            "xT": xT,
            "ones_self": np.ones((128, HPC, NJ - NJ_MEM, 1), dtype=np.float16),
            "wq": wq, "wk": wk, "wv": wv, "wo": wo,
            "mkT": mkT,
            "mv": np.ascontiguousarray(mv),
            "ebias": ebias,
        })
    return in_maps


def kernel(x, mem_k, mem_v, pos_bias, Wq, Wkv, Wo, bo, **_kw):
    nc = _build()
    in_maps = _prep_inputs(
        np.asarray(x), np.asarray(mem_k), np.asarray(mem_v),
        np.asarray(pos_bias), np.asarray(Wq), np.asarray(Wkv), np.asarray(Wo))
    res = bass_utils.run_bass_kernel_spmd(nc, in_maps, core_ids=list(range(NCORE)))
    out = np.zeros((B, N, DIM), dtype=np.float64)
    for r in res.results:
        out += r["out"].astype(np.float64)
    out += np.asarray(bo, dtype=np.float64)[None, None, :]
    return out.astype(np.float32)


# revision 36
# speedup vs baseline: 1.1432x; 1.1432x over previous
"""Trainium2 Bass kernel for memory-augmented causal attention.

Reference computation (b=2, n=1024, m=1024 memory, 16 heads, d_head=64):
  q = (x @ Wq) * scale ; k,v = split(x @ Wkv) ; k = [mem_k; k] ; v = [mem_v; v]
  sim = q k^T + pos_bias ; causal mask on self part ; softmax ; out = attn v
  return out @ Wo + bo

Sharding: 16 heads across 8 cores (2 heads/core), both batches on every core.
Each core computes a partial output (its heads' contribution through Wo rows);
host sums the 8 partials.

Softmax bias handling: exp(sim + bias) = exp(sim) * exp(bias).  The host
precomputes ebias = exp(pos_bias) with masked entries set to 0 (which also
applies the causal mask), f16.  ScalarE does a shifted exp of the raw logits
(PSUM->SBUF, exp(x-4) so masked-but-large logits can't hit f16 inf before
the zeroing multiply), then DVE multiplies by the ebias tile.

Softmax denominators come from a ones-column appended to V (row 64 of the
AV accumulation); all four (h, b) AV accumulators live in one [65, 2048]
PSUM tile so the whole normalization runs as a few wide DVE ops.
All matmuls are fp16 (full-rate on the TRN2 PE).
"""

import numpy as np

import concourse.bass as bass
import concourse.mybir as mybir
import concourse.tile as tile
from concourse import bacc
from concourse import bass_utils
from concourse.masks import make_identity

F32 = mybir.dt.float32
BF16 = mybir.dt.bfloat16
F16 = mybir.dt.float16

HEADS = 16
DH = 64               # head dim
B = 2                 # batch
N = 1024              # query length
M = 1024              # memory length
JT = N + M            # total key length
DIM = 1024
SCALE = DH ** -0.5
NCORE = 8
HPC = HEADS // NCORE  # heads per core = 2

NKC = DIM // 128      # contraction chunks for projections = 8
NJ = JT // 128        # j chunks = 16
NJ_MEM = M // 128     # memory j chunks = 8
NIC = N // 512        # i chunks of 512 = 2


def _self_chunks(ic):
    # self j-chunk k (j0 = 1024 + 128k) unmasked for i-chunk ic iff
    # j0 <= 1023 + ic*512 + 1024  ->  128k <= ic*512 + 511
    return (ic * 512 + 511) // 128 + 1


def _unmasked_jcs(ic):
    return list(range(NJ_MEM)) + [NJ_MEM + k for k in range(min(8, _self_chunks(ic)))]


_NC_CACHE = None


def _build():
    global _NC_CACHE
    if _NC_CACHE is not None:
        return _NC_CACHE

    nc = bacc.Bacc("TRN2", target_bir_lowering=False, debug=False)

    XT = nc.dram_tensor("xT", [B, DIM, N], F16, kind="ExternalInput").ap()
    WQ = nc.dram_tensor("wq", [DIM, 128], F16, kind="ExternalInput").ap()
    WK = nc.dram_tensor("wk", [DIM, 128], F16, kind="ExternalInput").ap()
    WV = nc.dram_tensor("wv", [DIM, 128], F16, kind="ExternalInput").ap()
    WO = nc.dram_tensor("wo", [128, DIM], F16, kind="ExternalInput").ap()
    MKT = nc.dram_tensor("mkT", [B, 128, M], F16, kind="ExternalInput").ap()
    MV = nc.dram_tensor("mv", [B, HPC, NJ_MEM, 128, DH + 1], F16,
                        kind="ExternalInput").ap()
    EBIAS = nc.dram_tensor("ebias", [HPC, JT, N], F16, kind="ExternalInput").ap()
    ONES = nc.dram_tensor("ones_self", [128, HPC, NJ - NJ_MEM, 1], F16,
                          kind="ExternalInput").ap()
    OUT = nc.dram_tensor("out", [B, N, DIM], F16, kind="ExternalOutput").ap()

    with tile.TileContext(nc) as tc:
        with tc.tile_pool(name="const", bufs=1) as cp, \
             tc.tile_pool(name="wts", bufs=1) as wp, \
             tc.tile_pool(name="xtp", bufs=16) as xtp, \
             tc.tile_pool(name="big", bufs=1) as bigp, \
             tc.tile_pool(name="stage", bufs=2) as stp, \
             tc.tile_pool(name="biasp", bufs=32) as biasp, \
             tc.tile_pool(name="expp", bufs=16) as expp, \
             tc.tile_pool(name="outst", bufs=4) as outstp, \
             tc.tile_pool(name="smalls", bufs=2) as smallp, \
             tc.tile_pool(name="psum", bufs=1, space="PSUM") as psp:

            # ---- constants ----
            identf = cp.tile([128, 128], F32)
            make_identity(nc, identf[:])
            identh = cp.tile([128, 128], F16)
            nc.vector.tensor_copy(identh[:], identf[:])
            # per-partition bias vector for the shifted exp
            negc = cp.tile([128, 1], F32, name="negc")
            nc.gpsimd.memset(negc[:], -4.0)

            # ---- weights (scalar queue; sync busy with xT) ----
            wq_t = wp.tile([128, NKC * 128], F16, tag="wqo")
            wk_t = wp.tile([128, NKC * 128], F16)
            wv_t = wp.tile([128, NKC * 128], F16)
            wo_t = wp.tile([128, DIM], F16, tag="wqo")
            def load_w(tl, src):
                nc.scalar.dma_start(
                    tl[:].rearrange("p (kc m) -> p kc m", m=128),
                    src.rearrange("(kc p) m -> p kc m", p=128))
            load_w(wq_t, WQ)

            # ---- persistent per-batch tensors ----
            qT = [bigp.tile([128, N], F16, name=f"qT{b}") for b in range(B)]
            kT = [bigp.tile([128, JT], F16, name=f"kT{b}") for b in range(B)]
            vaug = [bigp.tile([128, HPC * NJ * (DH + 1)], F16, name=f"vaug{b}")
                    for b in range(B)]

            def vaug_slice(b, h, jc):
                o = (h * NJ + jc) * (DH + 1)
                return vaug[b][:, o:o + DH + 1]
            outT = [bigp.tile([128, N], F16, name=f"outT{b}") for b in range(B)]

            def warm(n, width=128):
                wps = psp.tile([128, width], F32, name="warmps", tag="simps",
                               bufs=2)
                for _ in range(n):
                    nc.tensor.matmul(wps[:, 0:128], identh[:], identh[:],
                                     start=True, stop=True,
                                     skip_group_check=True)

            # preload all xT tiles for both batches (sync + scalar queues);
            xts = {}
            for b in range(B):
                for kc in range(NKC):
                    t = xtp.tile([128, N], F16, name=f"xt{b}_{kc}", tag="xt")
                    eng = nc.sync if (kc % 2 == 0) else nc.scalar
                    eng.dma_start(t[:], XT[b, kc * 128:(kc + 1) * 128, :])
                    xts[(b, kc)] = t
                if b == 0:
                    load_w(wk_t, WK)
            load_w(wv_t, WV)
            nc.scalar.dma_start(wo_t[:], WO)

            for b in range(B):
                # mem parts straight from DRAM
                nc.sync.dma_start(kT[b][:, 0:M], MKT[b])
                for h in range(HPC):
                    nc.gpsimd.dma_start(
                        vaug[b][:].rearrange(
                            "p (h jc x) -> p h jc x", h=HPC, x=DH + 1)[:, h, 0:NJ_MEM],
                        MV[b, h].rearrange("jc p x -> p jc x"))
                # ones columns for the self chunks, via strided DMA
                for h in range(HPC):
                    nc.gpsimd.dma_start(
                        vaug[b][:].rearrange(
                            "p (s x) -> p s x", x=DH + 1)[
                            :, h * NJ + NJ_MEM:h * NJ + NJ, DH:DH + 1],
                        ONES[:, h])

            # =============== Phase 1: projections ===============
            # ScalarE is idle until the first exp, so all projection-phase
            # PSUM evacuations go through it, keeping DVE free for the
            # attention-phase ebias multiplies.
            def proj_qk(kind, b):
                wt = wq_t if kind == "q" else wk_t
                ps = psp.tile([128, N], F32, name="projps", tag="simps", bufs=2)
                for icx in range(NIC):
                    for kc in range(NKC):
                        nc.tensor.matmul(
                            ps[:, icx * 512:(icx + 1) * 512],
                            wt[:, kc * 128:(kc + 1) * 128],
                            xts[(b, kc)][:, icx * 512:(icx + 1) * 512],
                            start=(kc == 0), stop=(kc == NKC - 1))
                if kind == "q":
                    nc.scalar.copy(qT[b][:], ps[:])
                else:
                    nc.scalar.copy(kT[b][:, M:JT], ps[:])

            def proj_v(b):
                vst = stp.tile([128, N], F16, name="vstage")
                ps = psp.tile([128, N], F32, name="vps", tag="simps", bufs=2)
                for icx in range(NIC):
                    for kc in range(NKC):
                        nc.tensor.matmul(
                            ps[:, icx * 512:(icx + 1) * 512],
                            wv_t[:, kc * 128:(kc + 1) * 128],
                            xts[(b, kc)][:, icx * 512:(icx + 1) * 512],
                            start=(kc == 0), stop=(kc == NKC - 1))
                nc.scalar.copy(vst[:], ps[:])
                tps = psp.tile([128, 8 * 128], F16, name="tps", tag="simps",
                               bufs=2)
                for jb in range(8):
                    nc.tensor.transpose(
                        tps[:, jb * 128:(jb + 1) * 128],
                        vst[:, jb * 128:(jb + 1) * 128], identh[:])
                # one fused copy after all 8 transposes (avoids PE-write /
                # ScalarE-read overlap on the shared PSUM bank)
                dst = vaug[b][:].rearrange(
                    "p (h jjc x) -> p jjc h x", h=HPC, x=DH + 1)[
                    :, NJ_MEM:NJ, :, 0:DH]
                src = tps[:].rearrange("p (jb h x) -> p jb h x", jb=8, x=DH)
                nc.scalar.copy(dst, src)

            warm(8)
            for kind, b in (("q", 0), ("q", 1), ("k", 0), ("k", 1)):
                proj_qk(kind, b)
            proj_v(0)
            proj_v(1)

            # =============== Phase 2: attention ===============
            # out-projection runs at the tail for both i-chunks: during
            # attention all 8 PSUM banks belong to the sims pipeline (4) and
            # the AV accumulators (4)
            opj_idx = [0]

            def out_proj_half(b, ib, allow_avps=True):
                ob = outstp.tile([128, DIM], F16, name="ob")
                for dc in range(DIM // 512):
                    # rotate across both psum tags (avh is dead by now) for
                    # effective depth 3: keeps the tail MM stream gap-free
                    opj_idx[0] += 1
                    if allow_avps and opj_idx[0] % 3 == 0:
                        ps = psp.tile([128, 512], F32, name="ops2",
                                      tag="avps", bufs=1)
                    else:
                        ps = psp.tile([128, 512], F32, name="ops", tag="simps",
                                      bufs=2)
                    nc.tensor.matmul(
                        ps[:],
                        outT[b][:, ib * 128:(ib + 1) * 128],
                        wo_t[:, dc * 512:(dc + 1) * 512],
                        start=True, stop=True)
                    if dc % 2 == 0:
                        nc.scalar.copy(ob[:, dc * 512:(dc + 1) * 512], ps[:])
                    else:
                        nc.vector.tensor_copy(ob[:, dc * 512:(dc + 1) * 512],
                                              ps[:])
                nc.sync.dma_start(OUT[b, ib * 128:(ib + 1) * 128, :], ob[:])

            epilogue = []  # deferred normalization work from the previous ic

            for ic in range(NIC):
                jcs = _unmasked_jcs(ic)
                avh = []  # single [65, 2048] accumulator, cols = (h, b, 512)
                pend = [[] for _ in range(HPC)]  # AV one iteration behind

                def do_av(h, p, last, avh=avh):
                    if not avh:
                        avh.append(psp.tile([DH + 1, B * HPC * 512], F32,
                                            name="av", tag="avps", bufs=1))
                    expt_, jc_, idx = p
                    for b in range(B):
                        o = (h * B + b) * 512
                        nc.tensor.matmul(
                            avh[0][:, o:o + 512],
                            vaug_slice(b, h, jc_),
                            expt_[:, b * 512:(b + 1) * 512],
                            start=(idx == 0), stop=last,
                            skip_group_check=True)

                for jj, jc in enumerate(jcs):
                    ebts = []
                    simps = []
                    for h in range(HPC):
                        ebt = biasp.tile([128, 512], F16, name=f"ebt{h}",
                                         tag="ebt")
                        # all ebias tiles on the sync queue: routing them via
                        # gpsimd couples them behind the (stalling)
                        # partition_broadcast in that FIFO and starves the AVs
                        nc.sync.dma_start(
                            ebt[:],
                            EBIAS[h, jc * 128:(jc + 1) * 128,
                                  ic * 512:(ic + 1) * 512])
                        ebts.append(ebt)
                        simps.append(psp.tile([128, N], F32, name=f"simps{h}",
                                              tag="simps", bufs=2))
                    # sims b-outer: consecutive MMs alternate 64-row groups
                    # (h0 at partitions 0-63, h1 at 64-127) so pairs can run
                    # concurrently as row-tiled matmuls
                    def sim_mm(b, h):
                        nc.tensor.matmul(
                            simps[h][:, b * 512:(b + 1) * 512],
                            kT[b][h * 64:(h + 1) * 64,
                                  jc * 128:(jc + 1) * 128],
                            qT[b][h * 64:(h + 1) * 64,
                                  ic * 512:(ic + 1) * 512],
                            start=True, stop=True, skip_group_check=True)
                    for b in range(B):
                        for h in range(HPC):
                            sim_mm(b, h)
                    # idempotent duplicates (start=True rewrites the same
                    # slice): filler PE work that closes the per-unit idle
                    # gaps so the HAM clock gate stays at 2.4 GHz
                    sim_mm(0, 0)
                    sim_mm(1, 1)
                    for h in range(HPC):
                        expr = expp.tile([128, N], F16, name="expr",
                                         tag="expr", bufs=3)
                        nc.scalar.activation(
                            expr[:], simps[h][:],
                            mybir.ActivationFunctionType.Exp, bias=negc[:])
                        expt = expp.tile([128, N], F16, name="expt",
                                         tag="expt")
                        for b in range(B):
                            nc.vector.tensor_tensor(
                                expt[:, b * 512:(b + 1) * 512],
                                expr[:, b * 512:(b + 1) * 512],
                                ebts[h][:], mybir.AluOpType.mult)
                        q = pend[h]
                        q.append((expt, jc, jj))
                        if jj == len(jcs) - 1:
                            # drain fully in the last unit so the ic boundary
                            # has no trailing AV burst stalling the next ic's
                            # sims (and thus the exp stream)
                            while q:
                                do_av(h, q.pop(0), len(q) == 0)
                        elif len(q) > 1:
                            do_av(h, q.pop(0), False)
                    if epilogue and jj >= 1:
                        # previous ic's deferred normalization: runs on DVE /
                        # gpsimd under the ScalarE-bound exp stream
                        epilogue.pop(0)()
                for h in range(HPC):
                    q = pend[h]
                    while q:
                        do_av(h, q.pop(0), len(q) == 0)

                def normalize(b, ic=ic, avh=avh):
                    # av columns are (h, b, 512); batch b's two head slices
                    # via a strided 3d AP so one chain normalizes them both
                    wide = HPC * 512

                    def bsl(src, p0, p1):
                        return src[p0:p1].rearrange(
                            "p (h b f) -> p b h f", h=HPC, b=B)[:, b]
                    sums_sb = smallp.tile([1, wide], F32, name="sums_sb")
                    nc.vector.tensor_copy(sums_sb[:].rearrange(
                        "p (h f) -> p h f", h=HPC), bsl(avh[0], DH, DH + 1))
                    recip = smallp.tile([1, wide], F32, name="recip")
                    nc.vector.reciprocal_approx_fast(recip[:], sums_sb[:])
                    recipb = smallp.tile([DH, wide], F32, name="recipb")
                    nc.gpsimd.partition_broadcast(recipb[:], recip[:])
                    nstage = smallp.tile([DH, wide], F16, name="nstage")
                    nc.vector.tensor_tensor(
                        nstage[:].rearrange("p (h f) -> p h f", h=HPC),
                        bsl(avh[0], 0, DH), recipb[:].rearrange(
                            "p (h f) -> p h f", h=HPC),
                        mybir.AluOpType.mult)
                    for h in range(HPC):
                        nc.sync.dma_start(
                            outT[b][h * 64:(h + 1) * 64,
                                    ic * 512:(ic + 1) * 512],
                            nstage[:, h * 512:(h + 1) * 512])

                if ic < NIC - 1:
                    epilogue.append(lambda f=normalize: f(0))
                    epilogue.append(lambda f=normalize: f(1))
                else:
                    # tail: normalize b0, project it while b1 normalizes
                    normalize(0)
                    normalize(1)

            # =============== Phase 3: output projection (tail) ===============
            for b in range(B):
                for ib in range(8):
                    out_proj_half(b, ib)

    nc.compile()
    _NC_CACHE = nc
    return nc


def _prep_inputs(x, mem_k, mem_v, pos_bias, Wq, Wkv, Wo):
    """Build per-core input maps (host-side sharding)."""
    x = np.ascontiguousarray(x, dtype=np.float32)
    xT = np.ascontiguousarray(x.transpose(0, 2, 1)).astype(np.float16)

    # exp(bias) with causal mask folded in as zeros: [16, JT, N] f16
    pb = np.ascontiguousarray(
        pos_bias[0].transpose(0, 2, 1)).astype(np.float32)     # [16, JT, N]
    jj = np.arange(JT)[:, None]
    ii = np.arange(N)[None, :]
    mask = jj > (ii + M)
    eb = np.where(mask[None], np.float32(0.0), np.exp(pb)).astype(np.float16)

    ones = np.ones((B, NJ_MEM, 128, 1), dtype=np.float16)
    in_maps = []
    for c in range(NCORE):
        cs = 128 * c
        wq = np.ascontiguousarray(Wq[:, cs:cs + 128] * SCALE).astype(np.float16)
        wk = np.ascontiguousarray(Wkv[:, cs:cs + 128]).astype(np.float16)
        wv = np.ascontiguousarray(Wkv[:, DIM + cs:DIM + cs + 128]).astype(np.float16)
        wo = np.ascontiguousarray(Wo[cs:cs + 128, :]).astype(np.float16)
        mkT = np.ascontiguousarray(
            mem_k[:, :, cs:cs + 128].transpose(0, 2, 1)).astype(np.float16)
        mv_s = mem_v[:, :, cs:cs + 128].astype(np.float16).reshape(B, NJ_MEM, 128, 2, DH)
        mv = np.empty((B, HPC, NJ_MEM, 128, DH + 1), dtype=np.float16)
        for h in range(HPC):
            mv[:, h, :, :, 0:DH] = mv_s[:, :, :, h, :]
            mv[:, h, :, :, DH:] = ones
        ebias = np.ascontiguousarray(eb[2 * c:2 * c + 2])
        in_maps.append({
            "xT": xT,
            "ones_self": np.ones((128, HPC, NJ - NJ_MEM, 1), dtype=np.float16),
            "wq": wq, "wk": wk, "wv": wv, "wo": wo,
            "mkT": mkT,
            "mv": np.ascontiguousarray(mv),
            "ebias": ebias,
        })
    return in_maps


def kernel(x, mem_k, mem_v, pos_bias, Wq, Wkv, Wo, bo, **_kw):
    nc = _build()
    in_maps = _prep_inputs(
        np.asarray(x), np.asarray(mem_k), np.asarray(mem_v),
        np.asarray(pos_bias), np.asarray(Wq), np.asarray(Wkv), np.asarray(Wo))
    res = bass_utils.run_bass_kernel_spmd(nc, in_maps, core_ids=list(range(NCORE)))
    out = np.zeros((B, N, DIM), dtype=np.float64)
    for r in res.results:
        out += r["out"].astype(np.float64)
    out += np.asarray(bo, dtype=np.float64)[None, None, :]
    return out.astype(np.float32)
